# revision 29
# baseline (speedup 1.0000x reference)
"""nn_LinearAttention Trainium2 kernel: head-parallel (2 heads/core, 8 cores),
chunked gated-delta-rule (C=128) with truncated UT-transform inverse.

v4: single fused pipeline. Gating/decay tables (elup, lamb, column scalars)
precomputed on host from the tiny a/b projections; z-projection sweeps and the
out-projection are interleaved into the chunk-recurrence pipeline so the PE
never idles (stays HAM-warm); PSUM banks repacked (5 banks for the recurrence,
2 rotating for z, 3 for out-proj after z retires); two-head ops merged where
layouts allow; elementwise work balanced across Vector/Scalar/GpSimd; output
DMA batched to 512KB descriptors.

Self-contained: builds one SPMD Bass program; host shards weights per core,
runs on 8 NeuronCores via run_bass_kernel_spmd, sums per-core partial outputs.
"""
import sys
import types
import numpy as np
import ml_dtypes

import concourse.bass as bass
import concourse.tile as tile
from concourse import mybir
from concourse.bass_utils import run_bass_kernel_spmd

F32 = mybir.dt.float32
BF16 = mybir.dt.float16  # 16-bit tile dtype: fp16 (same speed as bf16, finer mantissa)
AF = mybir.ActivationFunctionType
OP = mybir.AluOpType

H, DK, DV, HID, SEQ = 16, 64, 128, 2048, 2048
CH = 128                     # chunk length
NCH = SEQ // CH              # 16 chunks
NHID = HID // 128            # 16 hid tiles
NS4 = SEQ // 512             # 4 big s-chunks
NCOL = 2 * NCH
LN_QSCALE = -2.0794415416798357  # ln(1/8): folds q's 1/sqrt(DK) into exp


def _split_waits(nc, limit=1):
    """This container's walrus rejects >2 sync waits per instruction; Tile's
    final drain aggregates one wait per outstanding queue. Move extras onto
    carrier drains inserted just before."""
    f = nc.m.functions[0]
    for bb in f.blocks:
        out_insts, changed = [], False
        for inst in bb.instructions:
            si = inst.sync_info
            waits = list(si.on_wait) if si and si.on_wait else []
            if len(waits) > limit:
                changed = True
                extra, keep = waits[:-limit], waits[-limit:]
                for j, w in enumerate(extra):
                    out_insts.append(mybir.InstDrain(
                        name=f"{inst.name}-wsplit{j}", engine=inst.engine,
                        ins=[], outs=[],
                        sync_info=mybir.SyncInfo(on_wait=[w], on_update=[])))
                si.on_wait = keep
            out_insts.append(inst)
        if changed:
            bb.instructions = out_insts


def _make_consts(nc, pool):
    c = {}
    for name, dt in (("idf", F32), ("idb", BF16)):
        t = pool.tile([128, 128], dt, tag=name)
        nc.gpsimd.memset(t[:], 0.0)
        nc.gpsimd.affine_select(out=t[:], in_=t[:], compare_op=OP.not_equal,
                                fill=1.0, base=0, pattern=[[-1, 128]], channel_multiplier=1)
        c[name] = t
    # idb2 = [I | I] (both-head identity for merged (I - T) ops)
    i2 = pool.tile([128, 256], BF16, tag="idb2", name="idb2")
    nc.gpsimd.tensor_copy(i2[:, 0:128], c["idb"][:])
    nc.gpsimd.tensor_copy(i2[:, 128:256], c["idb"][:])
    c["idb2"] = i2
    ones_col_h = pool.tile([128, 1], BF16, tag="ones_col_h", name="ones_col_h")
    nc.gpsimd.memset(ones_col_h[:], 1.0)
    c["ones_col_h"] = ones_col_h
    ones_row = pool.tile([1, 128], BF16, tag="ones_row", name="ones_row")
    nc.gpsimd.memset(ones_row[:], 1.0)
    c["ones_row"] = ones_row
    qsc = pool.tile([2, 1], F32, tag="qsc", name="qsc")
    nc.gpsimd.memset(qsc[:], LN_QSCALE)
    c["qsc"] = qsc
    # ones_blk16[p, h] = 1 if p//64 == h   (head-block column selector, lhsT)
    ob = pool.tile([128, 2], BF16, tag="ones_blk", name="ones_blk")
    nc.gpsimd.memset(ob[:], 1.0)
    nc.gpsimd.affine_select(out=ob[:], in_=ob[:], compare_op=OP.is_ge,
                            fill=0.0, base=0, pattern=[[-64, 2]], channel_multiplier=1)
    nc.gpsimd.affine_select(out=ob[:], in_=ob[:], compare_op=OP.is_ge,
                            fill=0.0, base=63, pattern=[[64, 2]], channel_multiplier=-1)
    c["ones_blk"] = ob
    # sel2[h, f] = 1 if f//64 == h  (head-block row selector: bcast lhsT)
    s2 = pool.tile([2, 128], BF16, tag="sel2", name="sel2")
    nc.gpsimd.memset(s2[:], 1.0)
    nc.gpsimd.affine_select(out=s2[:], in_=s2[:], compare_op=OP.is_ge,
                            fill=0.0, base=0, pattern=[[1, 128]], channel_multiplier=-64)
    nc.gpsimd.affine_select(out=s2[:], in_=s2[:], compare_op=OP.is_ge,
                            fill=0.0, base=63, pattern=[[-1, 128]], channel_multiplier=64)
    c["sel2"] = s2
    return c


def _kernel_body(nc, tc, ctx, hsT, wqk, wvz, convw, wo, elup, lamb, colsc, out):
    from contextlib import ExitStack
    cpool = ctx.enter_context(tc.tile_pool(name="consts", bufs=1))
    C = _make_consts(nc, cpool)

    # ---- weight / input / table pools (DMA issue order matters: the sweep
    # stream paces behind the hst tiles, so those go right after wqk) ----
    wpoolP = ctx.enter_context(tc.tile_pool(name="wP", bufs=1))
    gt_pool = ctx.enter_context(tc.tile_pool(name="gtab", bufs=1))
    seqp = ctx.enter_context(tc.tile_pool(name="seqbufs", bufs=1))
    # kqT_all col = 256*n + 128*x + c, x=0 -> k, x=1 -> q (chunk-interleaved)
    kqT_all = seqp.tile([128, 2 * SEQ], BF16, tag="kqT", name="kqT")
    k_rows = seqp.tile([128, SEQ], BF16, tag="krows", name="krows")   # col = 128*n + 64h + dk
    kbl_all = seqp.tile([128, SEQ], BF16, tag="kbl", name="kbl")      # betaLam * k, same layout
    v_rows = seqp.tile([128, 2 * SEQ], BF16, tag="vrows", name="vrows")  # col = 256n + 128h + dv
    zT = [seqp.tile([128, SEQ], BF16, tag=f"zT{h}", name=f"zT{h}") for h in range(2)]
    OT_all = [seqp.tile([128, SEQ], BF16, tag=f"OT{h}", name=f"OT{h}") for h in range(2)]

    ioctx = ExitStack()   # wqk/wvz/convw/hst: released right after the z sweeps
    wpool = ioctx.enter_context(tc.tile_pool(name="wA", bufs=1))
    hstp = ioctx.enter_context(tc.tile_pool(name="hstp", bufs=1))

    wqk_sb = wpool.tile([128, NHID * 256], BF16, tag="wqk", name="wqk")
    nc.sync.dma_start(wqk_sb[:].rearrange("p (i c) -> p i c", i=NHID),
                      wqk.rearrange("(i p) c -> p i c", p=128))
    convw_sb = wpool.tile([128, 16], F32, tag="convw", name="convw")  # 4 groups x 4 taps
    nc.sync.dma_start(convw_sb[:].rearrange("p (g t) -> p g t", g=4),
                      convw.rearrange("(g p) t -> p g t", p=128))
    hst_all = hstp.tile([128, NHID * SEQ], BF16, tag="hst", name="hst")
    for i in range(NHID):
        nc.sync.dma_start(hst_all[:, SEQ * i:SEQ * (i + 1)],
                          hsT[128 * i:128 * i + 128, :])
    wvz_sb = wpool.tile([128, NHID * 512], BF16, tag="wvz", name="wvz")
    nc.sync.dma_start(wvz_sb[:].rearrange("p (i c) -> p i c", i=NHID),
                      wvz.rearrange("(i p) c -> p i c", p=128))
    colsc_sb = gt_pool.tile([128, 128], F32, tag="colsc", name="colsc")
    nc.sync.dma_start(colsc_sb[:], colsc)
    elup_sb = gt_pool.tile([128, NCOL * 256], BF16, tag="elup", name="elup")
    nc.sync.dma_start(elup_sb[:], elup)
    lamb_sb = gt_pool.tile([128, NCOL * 128], BF16, tag="lamb", name="lamb")
    nc.sync.dma_start(lamb_sb[:], lamb)
    wo_sb = [wpoolP.tile([128, HID], BF16, tag=f"wo{h}", name=f"wo{h}") for h in range(2)]
    for h in range(2):
        nc.sync.dma_start(wo_sb[h][:], wo[128 * h:128 * h + 128, :])
    BETA, BLAM, KTIL, LAMC = 0, 32, 64, 96   # column offsets inside colsc

    # ---------------- Phase A: q/k/v projections (K-contiguous sweeps) ----------------
    with tc.tile_pool(name="pA_ps", bufs=1, space="PSUM") as pA_ps, \
         tc.tile_pool(name="pA_mA", bufs=3, space="PSUM") as pA_mA, \
         tc.tile_pool(name="phaseA_sb", bufs=1) as pA:
        mx = [pA.tile([128, SEQ + 3], BF16, tag=f"mx{g}", name=f"mx{g}") for g in range(4)]
        for g in range(4):
            nc.vector.memset(mx[g][:, 0:3], 0.0)

        pss = [pA_ps.tile([128, 512], F32, tag=f"ps{s}", name=f"ps{s}")
               for s in range(NS4)]

        def sweep(wsl):
            """K-contiguous: for each K-tile i, 4 s-chunk matmuls into 4 fixed
            PSUM banks; stationary loaded once per i."""
            for i in range(NHID):
                w_ap = wsl(i)
                for s in range(NS4):
                    nc.tensor.matmul(pss[s][:], w_ap,
                                     hst_all[:, SEQ * i + 512 * s:SEQ * i + 512 * s + 512],
                                     start=(i == 0), stop=(i == NHID - 1))

        def evac_mx(g):
            for s in range(NS4):
                nc.scalar.copy(mx[g][:, 3 + 512 * s:3 + 512 * s + 512], pss[s][:])

        def conv_macs(g, s4):
            o = 512 * s4
            acc = pA.tile([128, 512], BF16, tag="acc", name="acc", bufs=3)
            nc.vector.tensor_scalar(acc[:], mx[g][:, o:o + 512],
                                    convw_sb[:, 4 * g:4 * g + 1], None, op0=OP.mult)
            for t in range(1, 4):
                nc.vector.scalar_tensor_tensor(acc[:], mx[g][:, o + t:o + t + 512],
                                               convw_sb[:, 4 * g + t:4 * g + t + 1],
                                               acc[:], op0=OP.mult, op1=OP.add)
            return acc

        # PE stream: q, k, v0, v1 sweeps back-to-back; conv/norm elementwise
        # work runs on V/S/G underneath the v sweeps.
        sweep(lambda i: wqk_sb[:, 256 * i:256 * i + 128])
        evac_mx(0)
        co_q, co_k = [], []
        for s4 in range(NS4):
            acc = conv_macs(0, s4)
            co = pA.tile([128, 512], BF16, tag=f"co0_{s4}", name="co", bufs=1)
            nc.scalar.activation(co[:], acc[:], AF.Silu)
            co_q.append(co)
        sweep(lambda i: wqk_sb[:, 256 * i + 128:256 * i + 256])
        evac_mx(1)
        for s4 in range(NS4):
            acc = conv_macs(1, s4)
            co = pA.tile([128, 512], BF16, tag=f"co1_{s4}", name="co", bufs=1)
            nc.scalar.activation(co[:], acc[:], AF.Silu)
            co_k.append(co)
        sweep(lambda i: wvz_sb[:, 512 * i:512 * i + 128])
        evac_mx(2)
        sweep(lambda i: wvz_sb[:, 512 * i + 128:512 * i + 256])
        evac_mx(3)

        # ---- qk l2-norm (ln_exp table set) ----
        for g, cos in ((0, co_q), (1, co_k)):
            ms = pA.tile([2, SEQ], F32, tag="ms", name="ms", bufs=1)
            rstd = pA.tile([2, SEQ], BF16, tag="rstd", name="rstd", bufs=1)
            for s4 in range(NS4):
                sq = pA.tile([128, 512], BF16, tag="sq", name="sq", bufs=2)
                nc.gpsimd.tensor_tensor(sq[:], cos[s4][:], cos[s4][:], op=OP.mult)
                nrm = pA_mA.tile([128, 512], F32, tag="mA", name="mA")
                nc.tensor.matmul(nrm[0:2, :], C["ones_blk"][:], sq[:], start=True, stop=True)
                nc.vector.tensor_scalar(ms[:, 512 * s4:512 * s4 + 512], nrm[0:2, :],
                                        1e-6, None, op0=OP.add)
            nc.scalar.activation(ms[:], ms[:], AF.Ln)
            if g == 0:
                nc.scalar.activation(rstd[:], ms[:], AF.Exp, scale=-0.5, bias=C["qsc"][:])
            else:
                nc.scalar.activation(rstd[:], ms[:], AF.Exp, scale=-0.5)
            # normalize-mult into kqT_all while tiles live (x=1 for q, 0 for k)
            x = 1 - g
            kq4 = kqT_all[:].rearrange("p (n x c) -> p n x c", x=2, c=128)
            for s4 in range(NS4):
                bc = pA_mA.tile([128, 512], F32, tag="mA", name="mA")
                nc.tensor.matmul(bc[:], C["sel2"][:], rstd[:, 512 * s4:512 * s4 + 512],
                                 start=True, stop=True)
                nc.vector.tensor_tensor(
                    kq4[:, 4 * s4:4 * s4 + 4, x, :],
                    bc[:].rearrange("p (t c) -> p t c", c=128),
                    cos[s4][:].rearrange("p (t c) -> p t c", c=128), op=OP.mult)
        for s4 in range(NS4):  # k row layout
            kt = pA_mA.tile([128, 512], BF16, tag="mA", name="mA")
            for j in range(4):
                nn = 4 * s4 + j
                nc.tensor.transpose(kt[:, 128 * j:128 * j + 128],
                                    kqT_all[:, 256 * nn:256 * nn + 128], C["idb"][:])
            nc.scalar.copy(k_rows[:, 512 * s4:512 * s4 + 512], kt[:])
            for j in range(4):
                n = 4 * s4 + j
                for h in range(2):
                    col = 2 * n + h
                    sl = slice(128 * n + 64 * h, 128 * n + 64 * h + 64)
                    nc.vector.tensor_scalar(kbl_all[:, sl], k_rows[:, sl],
                                            colsc_sb[:, BLAM + col:BLAM + col + 1],
                                            None, op0=OP.mult)
            for j in range(4):
                n = 4 * s4 + j
                for h in range(2):
                    col = 2 * n + h
                    sl = slice(128 * n + 64 * h, 128 * n + 64 * h + 64)
                    nc.vector.tensor_scalar(k_rows[:, sl], k_rows[:, sl],
                                            colsc_sb[:, KTIL + col:KTIL + col + 1],
                                            None, op0=OP.mult)

        # ---- v conv (silu) + transpose to row layout ----
        vr = v_rows[:].rearrange("p (t x c) -> p t x c", t=16, x=2)
        for g in (2, 3):
            h = g - 2
            for s4 in range(NS4):
                acc = conv_macs(g, s4)
                co = pA.tile([128, 512], BF16, tag="cov", name="cov", bufs=2)
                nc.scalar.activation(co[:], acc[:], AF.Silu)
                pt = pA_mA.tile([128, 512], BF16, tag="mA", name="mA")
                for j in range(4):
                    nc.tensor.transpose(pt[:, 128 * j:128 * j + 128],
                                        co[:, 128 * j:128 * j + 128], C["idb"][:])
                nc.scalar.copy(vr[:, 4 * s4:4 * s4 + 4, h, :],
                               pt[:].rearrange("p (j c) -> p j c", j=4))
                for j in range(4):
                    n = 4 * s4 + j
                    col = 2 * n + h
                    sl = slice(256 * n + 128 * h, 256 * n + 128 * h + 128)
                    nc.vector.tensor_scalar(v_rows[:, sl], v_rows[:, sl],
                                            colsc_sb[:, BETA + col:BETA + col + 1],
                                            None, op0=OP.mult)

    # ---------------- z sweeps (own PSUM pool, before the chunk pipeline) ----------------
    with tc.tile_pool(name="zp", bufs=1, space="PSUM") as zpool:
        zps = [zpool.tile([128, 512], F32, tag=f"zps{s}", name=f"zps{s}")
               for s in range(2)]
        for (h, sblk) in ((0, 0), (1, 0), (0, 1), (1, 1)):
            for i in range(NHID):
                for s in range(2):
                    blk = 2 * sblk + s
                    nc.tensor.matmul(
                        zps[s][:], wvz_sb[:, 512 * i + 256 + 128 * h:512 * i + 384 + 128 * h],
                        hst_all[:, SEQ * i + 512 * blk:SEQ * i + 512 * blk + 512],
                        start=(i == 0), stop=(i == NHID - 1))
                if i == NHID - 1:
                    for s in range(2):
                        blk = 2 * sblk + s
                        nc.scalar.activation(zT[h][:, 512 * blk:512 * blk + 512],
                                             zps[s][:], AF.Silu)
    ioctx.close()   # free hst/wqk/wvz/convw SBUF for the chunk pipeline

    # ---------------- Phase B: chunks, software-pipelined (v3 structure) ----------------
    sbp = ctx.enter_context(tc.tile_pool(name="chunk_sb", bufs=1))
    stp = ctx.enter_context(tc.tile_pool(name="state", bufs=2))
    gpP = ctx.enter_context(tc.tile_pool(name="gating", bufs=1))
    S_sb = [stp.tile([64, 128], BF16, tag=f"S{h}", name=f"S{h}") for h in range(2)]
    for h in range(2):
        nc.vector.memset(S_sb[h][:], 0.0)

    st = {}  # (n, h) -> dict of tiles

    gated = {}  # s4 -> {h: gt}
    with tc.tile_pool(name="pB", bufs=1, space="PSUM") as pB, \
         tc.tile_pool(name="pC", bufs=2, space="PSUM") as pC:
        bank1 = [pB.tile([128, 512], F32, tag=f"bank1_{h}", name=f"bank1_{h}")
                 for h in range(2)]
        bank2 = [pB.tile([128, 512], F32, tag=f"bank2_{h}", name=f"bank2_{h}")
                 for h in range(2)]
        serT = pB.tile([128, 512], F32, tag="ser", name="ser")
        ser = [serT[:, 0:256], serT[:, 256:512]]   # per-head halves
        ptr2 = pB.tile([128, 256], BF16, tag="ptr2", name="ptr2")
        ptrs = [ptr2[:, 0:128], ptr2[:, 128:256]]

        def s1(n, h):
            col = 2 * n + h
            d = st[(n, h)] = {}
            kTs = kqT_all[64 * h:64 * h + 64, 256 * n:256 * n + 128]
            kqs = kqT_all[64 * h:64 * h + 64, 256 * n:256 * n + 256]
            psg = bank1[h][:, 0:256]
            nc.tensor.matmul(psg, kTs, kqs, start=True, stop=True)
            d["psg"] = psg
            rhs = sbp.tile([128, 192], BF16, tag=f"rhs{h}", name="rhs", bufs=6)
            nc.gpsimd.tensor_copy(rhs[:, 0:64],
                                  kbl_all[:, 128 * n + 64 * h:128 * n + 64 * h + 64])
            nc.gpsimd.tensor_copy(rhs[:, 64:192],
                                  v_rows[:, 256 * n + 128 * h:256 * n + 128 * h + 128])
            d["rhs"] = rhs

        def s2(n, h):
            col = 2 * n + h
            d = st[(n, h)]
            amtk = sbp.tile([128, 320], BF16, tag=f"amtk{h}", name="amtk", bufs=8)
            nc.vector.tensor_tensor(amtk[:, 0:256], d["psg"][:],
                                    elup_sb[:, 256 * col:256 * col + 256], op=OP.mult)
            nc.gpsimd.tensor_copy(amtk[:, 256:320],
                                   k_rows[:, 128 * n + 64 * h:128 * n + 64 * h + 64])
            d["amtk"] = amtk
            nc.tensor.transpose(ptrs[h][:], amtk[:, 0:128], C["idb"][:])
            d["ptr"] = ptrs[h]

        def s3a(n, h):
            d = st[(n, h)]
            Bsb = sbp.tile([128, 128], BF16, tag=f"Bsb{h}", name="Bsb", bufs=4)
            nc.scalar.copy(Bsb[:], d["ptr"][:])
            Psb = sbp.tile([128, 128], BF16, tag=f"Psb{h}", name="Psb", bufs=4)
            nc.vector.tensor_tensor(Psb[:], C["idb"][:], d["ptr"][:], op=OP.subtract)
            psq = bank1[h][:, 256:384]
            nc.tensor.matmul(psq, d["amtk"][:, 0:128], Bsb[:], start=True, stop=True)
            d["Psb"], d["psq"] = Psb, psq

        def s3b(n, h):
            d = st[(n, h)]
            P1 = sbp.tile([128, 128], BF16, tag=f"P1{h}", name="P1", bufs=4)
            nc.vector.tensor_tensor(P1[:], d["Psb"][:], d["psq"], op=OP.add)
            pwu = bank2[h][:, 0:192]
            nc.tensor.matmul(pwu, P1[:], d["rhs"][:], start=True, stop=True)
            wu = sbp.tile([128, 192], BF16, tag=f"wu{h}", name="wu", bufs=6)
            if h == 0:
                nc.vector.tensor_copy(wu[:], pwu)
            else:
                nc.scalar.copy(wu[:], pwu)
            d["wu"] = wu

        def s4a(n, h):
            col = 2 * n + h
            d = st[(n, h)]
            psm = bank2[h][0:64, 192:384]
            nc.tensor.matmul(psm, d["wu"][:, 0:64], d["amtk"][:, 128:320],
                             start=True, stop=True)
            qlam = sbp.tile([64, 128], BF16, tag=f"qlam{h}", name="qlam", bufs=4)
            nc.gpsimd.tensor_tensor(
                qlam[:], lamb_sb[64 * h:64 * h + 64, 128 * col:128 * col + 128],
                kqT_all[64 * h:64 * h + 64, 256 * n + 128:256 * n + 256], op=OP.mult)
            d["psm"], d["qlam"] = psm, qlam

        def s4b(n, h):
            col = 2 * n + h
            d = st[(n, h)]
            Pt = sbp.tile([64, 128], BF16, tag=f"Pt{h}", name="Pt", bufs=4)
            nc.vector.tensor_tensor(Pt[:], d["qlam"][:], d["psm"][:, 0:128], op=OP.subtract)
            GhT = sbp.tile([64, 64], BF16, tag=f"GhT{h}", name="GhT", bufs=4)
            nc.vector.scalar_tensor_tensor(GhT[:], C["idf"][0:64, 0:64],
                                           colsc_sb[0:64, LAMC + col:LAMC + col + 1],
                                           d["psm"][:, 128:192],
                                           op0=OP.mult, op1=OP.subtract)
            pot = ser[h][:, 0:128]
            nc.tensor.matmul(pot, S_sb[h][:], Pt[:], start=True, stop=False)
            nc.tensor.matmul(pot, d["wu"][:, 64:192], d["amtk"][:, 128:256],
                             start=False, stop=True)
            if h == 0:
                nc.vector.tensor_copy(OT_all[h][:, CH * n:CH * n + CH], pot)
            else:
                nc.scalar.copy(OT_all[h][:, CH * n:CH * n + CH], pot)
            pst = ser[h][0:64, 128:256]
            nc.tensor.matmul(pst, GhT[:], S_sb[h][:], start=True, stop=False)
            nc.tensor.matmul(pst, d["amtk"][:, 256:320], d["wu"][:, 64:192],
                             start=False, stop=True)
            Snew = stp.tile([64, 128], BF16, tag=f"S{h}", name=f"S{h}")
            nc.scalar.copy(Snew[:], pst)
            S_sb[h] = Snew
            del st[(n, h)]

        def c_prep(s4):
            sl = slice(512 * s4, 512 * s4 + 512)
            ms4 = gpP.tile([1, 1024], F32, tag="ms4", name="ms4", bufs=2)
            rstd4 = gpP.tile([1, 1024], BF16, tag="rstd4", name="rstd4", bufs=2)
            for h in range(2):
                sq = gpP.tile([128, 512], BF16, tag="sq", name="sq", bufs=2)
                nc.gpsimd.tensor_tensor(sq[:], OT_all[h][:, sl], OT_all[h][:, sl],
                                        op=OP.mult)
                pn = pC.tile([128, 512], F32, tag="pc", name="pn")
                nc.tensor.matmul(pn[0:1, :], C["ones_col_h"][:], sq[:],
                                 start=True, stop=True)
                nc.vector.tensor_scalar(ms4[:, 512 * h:512 * h + 512], pn[0:1, :],
                                        1.0 / DV, 1e-6, op0=OP.mult, op1=OP.add)
            nc.scalar.activation(ms4[:], ms4[:], AF.Ln)
            nc.scalar.activation(rstd4[:], ms4[:], AF.Exp, scale=-0.5)
            gated[s4] = {}
            for h in range(2):
                pb = pC.tile([128, 512], F32, tag="pc", name="pb")
                nc.tensor.matmul(pb[:], C["ones_row"][:], rstd4[:, 512 * h:512 * h + 512],
                                 start=True, stop=True)
                gt = gpP.tile([128, 512], BF16, tag=f"gt{h}", name="gt", bufs=2)
                nc.vector.tensor_tensor(gt[:], OT_all[h][:, sl], pb[:], op=OP.mult)
                nc.gpsimd.tensor_tensor(gt[:], gt[:], zT[h][:, sl], op=OP.mult)
                gated[s4][h] = gt

        def c_po(s4, j):
            s = 4 * s4 + j
            ot = gpP.tile([128, 2048], BF16, tag="ot", name="ot", bufs=3)
            for ho in range(4):
                po = pC.tile([128, 512], F32, tag="pc", name="po")
                for h in range(2):
                    nc.tensor.matmul(po[:], gated[s4][h][:, 128 * j:128 * j + 128],
                                     wo_sb[h][:, 512 * ho:512 * ho + 512],
                                     start=(h == 0), stop=(h == 1))
                if ho % 2 == 0:
                    nc.vector.tensor_copy(ot[:, 512 * ho:512 * ho + 512], po[:])
                else:
                    nc.scalar.copy(ot[:, 512 * ho:512 * ho + 512], po[:])
            nc.sync.dma_start(out[128 * s:128 * s + 128, :], ot[:])
            if j == 3:
                del gated[s4]

        cwork = []
        for s4 in range(NS4):
            cwork.append(lambda s4=s4: c_prep(s4))
            for j in range(4):
                cwork.append(lambda s4=s4, j=j: c_po(s4, j))
        cwork.reverse()  # pop() from the end

        stages = (s4b, s4a, s3b, s3a, s2, s1)
        for t in range(NCH + len(stages) - 1):
            for k, stage in enumerate(stages):
                n = t - (len(stages) - 1 - k)
                if 0 <= n < NCH:
                    for h in range(2):
                        stage(n, h)
            # drain out-proj pieces as their OT blocks land (OT(4*s4+3) at tick 4*s4+8)
            if t >= 9:
                ready_until = (t - 8) // 4
                budget = 2
                while budget and cwork and (5 * NS4 - len(cwork)) // 5 <= ready_until:
                    cwork.pop()()
                    budget -= 1
        while cwork:
            cwork.pop()()


def _build_program():
    from contextlib import ExitStack
    nc = bass.Bass("TRN2", target_bir_lowering=False, debug=False)
    hsT = nc.dram_tensor("hsT", [HID, SEQ], BF16, kind="ExternalInput").ap()
    wqk = nc.dram_tensor("wqk", [HID, 256], BF16, kind="ExternalInput").ap()
    wvz = nc.dram_tensor("wvz", [HID, 512], BF16, kind="ExternalInput").ap()
    convw = nc.dram_tensor("convw", [512, 4], F32, kind="ExternalInput").ap()
    wo = nc.dram_tensor("wo", [256, HID], BF16, kind="ExternalInput").ap()
    elup = nc.dram_tensor("elup", [128, NCOL * 256], BF16, kind="ExternalInput").ap()
    lamb = nc.dram_tensor("lamb", [128, NCOL * 128], BF16, kind="ExternalInput").ap()
    colsc = nc.dram_tensor("colsc", [128, 128], F32, kind="ExternalInput").ap()
    out = nc.dram_tensor("out", [SEQ, HID], BF16, kind="ExternalOutput").ap()
    with tile.TileContext(nc) as tc:
        with ExitStack() as ctx:
            _kernel_body(nc, tc, ctx, hsT, wqk, wvz, convw, wo, elup, lamb, colsc, out)
    _split_waits(nc)
    return nc


_PROG = None


def _get_program():
    global _PROG
    if _PROG is None:
        _PROG = _build_program()
    return _PROG


def _shim_ntff_hook():
    """Make bass_utils' `from antenv.axon_hooks import ...` importable."""
    if "antenv.axon_hooks" in sys.modules:
        return
    try:
        import trn_agent_boot.trn_boot as tb
        hook = tb._ntff_profile_via_ctypes("/opt/axon/libaxon_pjrt.so")
    except Exception:
        hook = None
    m = types.ModuleType("antenv.axon_hooks")
    m.get_axon_ntff_profile_hook = lambda: hook
    sys.modules["antenv.axon_hooks"] = m


def _softplus(x):
    return np.logaddexp(0.0, x)


def make_core_inputs(hidden_states, in_proj_qkv, in_proj_a, in_proj_b, in_proj_z,
                     conv_w, A_log, dt_bias, norm_w, out_proj):
    """Host-side sharding: per-core input dicts (core c owns heads 2c, 2c+1).
    Also precomputes, per (chunk, head), the gating/decay tables:
      elup: [A_lower | U_upper] 128x256 blocks (attention-decay matrices)
      lamb: exp(b_j) broadcast rows (128 x 128 per block)
      colsc: per-position column scalars [beta | beta*exp(b) | exp(bC - b) | exp(bC)]
    """
    hs = np.asarray(hidden_states, np.float32)[0]          # (S, HID)
    qkvT = np.ascontiguousarray(np.asarray(in_proj_qkv, np.float32).T)  # (HID, CONV)
    zTw = np.asarray(in_proj_z, np.float32).T              # (HID, VAL)
    cw = np.asarray(conv_w, np.float32)[:, 0, :]           # (CONV, 4)
    A_log = np.asarray(A_log, np.float32)
    dt_bias = np.asarray(dt_bias, np.float32)
    norm_w = np.asarray(norm_w, np.float32)
    op = np.asarray(out_proj, np.float32)                  # (HID, VAL)

    # tiny a/b projections + all decay tables, in float64 on host
    hs64 = hs.astype(np.float64)
    a_full = hs64 @ np.asarray(in_proj_a, np.float64).T    # (S, H)
    b_full = hs64 @ np.asarray(in_proj_b, np.float64).T
    g_full = -np.exp(A_log.astype(np.float64)) * _softplus(a_full + dt_bias)  # (S, H)
    beta_full = 1.0 / (1.0 + np.exp(-b_full))              # (S, H)
    # per-chunk inclusive cumsum of g
    gc = g_full.reshape(NCH, CH, H)
    bcum = np.cumsum(gc, axis=1)                           # (NCH, CH, H)
    betac = beta_full.reshape(NCH, CH, H)

    hsT = np.ascontiguousarray(hs.T).astype(np.float16)    # (HID, S) shared
    pos = np.arange(CH)
    low_mask = pos[:, None] > pos[None, :]                 # j < p strict
    up_mask = pos[:, None] <= pos[None, :]                 # j >= p
    maps = []
    for c in range(8):
        h0, h1 = 2 * c, 2 * c + 1
        qcols = list(range(64 * h0, 64 * h0 + 64)) + list(range(64 * h1, 64 * h1 + 64))
        kcols = [1024 + i for i in qcols]
        vcols0 = list(range(2048 + 128 * h0, 2048 + 128 * h0 + 128))
        vcols1 = list(range(2048 + 128 * h1, 2048 + 128 * h1 + 128))
        wqk = np.ascontiguousarray(qkvT[:, qcols + kcols]).astype(np.float16)
        wvz = np.ascontiguousarray(np.concatenate(
            [qkvT[:, vcols0], qkvT[:, vcols1], zTw[:, 128 * h0:128 * h0 + 128],
             zTw[:, 128 * h1:128 * h1 + 128]], axis=1)).astype(np.float16)
        convw = np.ascontiguousarray(np.concatenate(
            [cw[qcols], cw[kcols], cw[vcols0[0] - 2048 + 2048:vcols0[-1] - 2048 + 2049],
             cw[vcols1[0]:vcols1[-1] + 1]], axis=0))
        wo = np.ascontiguousarray(np.concatenate(
            [op[:, 128 * h0:128 * h0 + 128].T * norm_w[:, None],
             op[:, 128 * h1:128 * h1 + 128].T * norm_w[:, None]],
            axis=0)).astype(np.float16)

        elup = np.zeros((128, NCOL * 256), np.float64)
        lamb = np.zeros((128, NCOL * 128), np.float64)
        colsc = np.zeros((128, 128), np.float64)
        for n in range(NCH):
            for hh, hg in ((0, h0), (1, h1)):
                col = 2 * n + hh
                b = bcum[n, :, hg]                          # (128,)
                beta = betac[n, :, hg]
                # A_lower[p, j] = beta_p * exp(b_p - b_j) for j < p
                # (b decreasing: kept region has b_p - b_j <= 0; clamp the rest)
                A_l = beta[:, None] * np.exp(np.minimum(b[:, None] - b[None, :], 0.0)) * low_mask
                # U_upper[p, j] = exp(b_j - b_p) for j >= p
                U_u = np.exp(np.minimum(b[None, :] - b[:, None], 0.0)) * up_mask
                elup[:, 256 * col:256 * col + 128] = A_l
                elup[:, 256 * col + 128:256 * col + 256] = U_u
                lamb[:, 128 * col:128 * col + 128] = np.exp(b)[None, :]
                colsc[:, col] = beta
                colsc[:, 32 + col] = beta * np.exp(b)
                colsc[:, 64 + col] = np.exp(b[-1] - b)
                colsc[:, 96 + col] = np.exp(b[-1])
        maps.append({"hsT": hsT, "wqk": wqk, "wvz": wvz, "convw": convw, "wo": wo,
                     "elup": elup.astype(np.float16),
                     "lamb": lamb.astype(np.float16),
                     "colsc": colsc.astype(np.float32)})
    return maps


def kernel(hidden_states, in_proj_qkv, in_proj_a, in_proj_b, in_proj_z,
           conv_w, A_log, dt_bias, norm_w, out_proj, is_prefill=1, **_ignored):
    _shim_ntff_hook()
    nc = _get_program()
    maps = make_core_inputs(hidden_states, in_proj_qkv, in_proj_a, in_proj_b,
                            in_proj_z, conv_w, A_log, dt_bias, norm_w, out_proj)
    res = run_bass_kernel_spmd(nc, maps, core_ids=list(range(8)))
    acc = res.results[0]["out"].astype(np.float32)
    for i in range(1, 8):
        acc += res.results[i]["out"].astype(np.float32)
    return acc[None, :, :]


# revision 31
# speedup vs baseline: 1.0585x; 1.0585x over previous
"""nn_LinearAttention Trainium2 kernel: head-parallel (2 heads/core, 8 cores),
chunked gated-delta-rule (C=128) with truncated UT-transform inverse.

v4: single fused pipeline. Gating/decay tables (elup, lamb, column scalars)
precomputed on host from the tiny a/b projections; z-projection sweeps and the
out-projection are interleaved into the chunk-recurrence pipeline so the PE
never idles (stays HAM-warm); PSUM banks repacked (5 banks for the recurrence,
2 rotating for z, 3 for out-proj after z retires); two-head ops merged where
layouts allow; elementwise work balanced across Vector/Scalar/GpSimd; output
DMA batched to 512KB descriptors.

Self-contained: builds one SPMD Bass program; host shards weights per core,
runs on 8 NeuronCores via run_bass_kernel_spmd, sums per-core partial outputs.
"""
import sys
import types
import numpy as np
import ml_dtypes

import concourse.bass as bass
import concourse.tile as tile
from concourse import mybir
from concourse.bass_utils import run_bass_kernel_spmd

F32 = mybir.dt.float32
BF16 = mybir.dt.float16  # 16-bit tile dtype: fp16 (same speed as bf16, finer mantissa)
AF = mybir.ActivationFunctionType
OP = mybir.AluOpType

H, DK, DV, HID, SEQ = 16, 64, 128, 2048, 2048
CH = 128                     # chunk length
NCH = SEQ // CH              # 16 chunks
NHID = HID // 128            # 16 hid tiles
NS4 = SEQ // 512             # 4 big s-chunks
NCOL = 2 * NCH
LN_QSCALE = -2.0794415416798357  # ln(1/8): folds q's 1/sqrt(DK) into exp


def _split_waits(nc, limit=1):
    """This container's walrus rejects >2 sync waits per instruction; Tile's
    final drain aggregates one wait per outstanding queue. Move extras onto
    carrier drains inserted just before."""
    f = nc.m.functions[0]
    for bb in f.blocks:
        out_insts, changed = [], False
        for inst in bb.instructions:
            si = inst.sync_info
            waits = list(si.on_wait) if si and si.on_wait else []
            if len(waits) > limit:
                changed = True
                extra, keep = waits[:-limit], waits[-limit:]
                for j, w in enumerate(extra):
                    out_insts.append(mybir.InstDrain(
                        name=f"{inst.name}-wsplit{j}", engine=inst.engine,
                        ins=[], outs=[],
                        sync_info=mybir.SyncInfo(on_wait=[w], on_update=[])))
                si.on_wait = keep
            out_insts.append(inst)
        if changed:
            bb.instructions = out_insts


def _make_consts(nc, pool):
    c = {}
    for name, dt in (("idf", F32), ("idb", BF16)):
        t = pool.tile([128, 128], dt, tag=name)
        nc.gpsimd.memset(t[:], 0.0)
        nc.gpsimd.affine_select(out=t[:], in_=t[:], compare_op=OP.not_equal,
                                fill=1.0, base=0, pattern=[[-1, 128]], channel_multiplier=1)
        c[name] = t
    # idb2 = [I | I] (both-head identity for merged (I - T) ops)
    i2 = pool.tile([128, 256], BF16, tag="idb2", name="idb2")
    nc.gpsimd.tensor_copy(i2[:, 0:128], c["idb"][:])
    nc.gpsimd.tensor_copy(i2[:, 128:256], c["idb"][:])
    c["idb2"] = i2
    ones_col_h = pool.tile([128, 1], BF16, tag="ones_col_h", name="ones_col_h")
    nc.gpsimd.memset(ones_col_h[:], 1.0)
    c["ones_col_h"] = ones_col_h
    ones_row = pool.tile([1, 128], BF16, tag="ones_row", name="ones_row")
    nc.gpsimd.memset(ones_row[:], 1.0)
    c["ones_row"] = ones_row
    qsc = pool.tile([2, 1], F32, tag="qsc", name="qsc")
    nc.gpsimd.memset(qsc[:], LN_QSCALE)
    c["qsc"] = qsc
    # ones_blk16[p, h] = 1 if p//64 == h   (head-block column selector, lhsT)
    ob = pool.tile([128, 2], BF16, tag="ones_blk", name="ones_blk")
    nc.gpsimd.memset(ob[:], 1.0)
    nc.gpsimd.affine_select(out=ob[:], in_=ob[:], compare_op=OP.is_ge,
                            fill=0.0, base=0, pattern=[[-64, 2]], channel_multiplier=1)
    nc.gpsimd.affine_select(out=ob[:], in_=ob[:], compare_op=OP.is_ge,
                            fill=0.0, base=63, pattern=[[64, 2]], channel_multiplier=-1)
    c["ones_blk"] = ob
    # sel2[h, f] = 1 if f//64 == h  (head-block row selector: bcast lhsT)
    s2 = pool.tile([2, 128], BF16, tag="sel2", name="sel2")
    nc.gpsimd.memset(s2[:], 1.0)
    nc.gpsimd.affine_select(out=s2[:], in_=s2[:], compare_op=OP.is_ge,
                            fill=0.0, base=0, pattern=[[1, 128]], channel_multiplier=-64)
    nc.gpsimd.affine_select(out=s2[:], in_=s2[:], compare_op=OP.is_ge,
                            fill=0.0, base=63, pattern=[[-1, 128]], channel_multiplier=64)
    c["sel2"] = s2
    return c


def _kernel_body(nc, tc, ctx, hsT, wqk, wvz, convw, wo, elup, lamb, colsc, out):
    from contextlib import ExitStack
    cpool = ctx.enter_context(tc.tile_pool(name="consts", bufs=1))
    C = _make_consts(nc, cpool)

    # ---- weight / input / table pools (DMA issue order matters: the sweep
    # stream paces behind the hst tiles, so those go right after wqk) ----
    wpoolP = ctx.enter_context(tc.tile_pool(name="wP", bufs=1))
    gt_pool = ctx.enter_context(tc.tile_pool(name="gtab", bufs=1))
    seqp = ctx.enter_context(tc.tile_pool(name="seqbufs", bufs=1))
    # kqT_all col = 256*n + 128*x + c, x=0 -> k, x=1 -> q (chunk-interleaved)
    kqT_all = seqp.tile([128, 2 * SEQ], BF16, tag="kqT", name="kqT")
    k_rows = seqp.tile([128, SEQ], BF16, tag="krows", name="krows")   # col = 128*n + 64h + dk
    v_rows = seqp.tile([128, 2 * SEQ], BF16, tag="vrows", name="vrows")  # col = 256n + 128h + dv
    zT = [seqp.tile([128, SEQ], BF16, tag=f"zT{h}", name=f"zT{h}") for h in range(2)]
    OT_all = [seqp.tile([128, SEQ], BF16, tag=f"OT{h}", name=f"OT{h}") for h in range(2)]

    ioctx = ExitStack()   # wqk/wvz/convw/hst: released right after the z sweeps
    wpool = ioctx.enter_context(tc.tile_pool(name="wA", bufs=1))
    hstp = ioctx.enter_context(tc.tile_pool(name="hstp", bufs=1))

    wqk_sb = wpool.tile([128, NHID * 256], BF16, tag="wqk", name="wqk")
    nc.sync.dma_start(wqk_sb[:].rearrange("p (i c) -> p i c", i=NHID),
                      wqk.rearrange("(i p) c -> p i c", p=128))
    convw_sb = wpool.tile([128, 16], F32, tag="convw", name="convw")  # 4 groups x 4 taps
    nc.sync.dma_start(convw_sb[:].rearrange("p (g t) -> p g t", g=4),
                      convw.rearrange("(g p) t -> p g t", p=128))
    hst_all = hstp.tile([128, NHID * SEQ], BF16, tag="hst", name="hst")
    for i in range(NHID):
        nc.sync.dma_start(hst_all[:, SEQ * i:SEQ * (i + 1)],
                          hsT[128 * i:128 * i + 128, :])
    wvz_sb = wpool.tile([128, NHID * 512], BF16, tag="wvz", name="wvz")
    nc.sync.dma_start(wvz_sb[:].rearrange("p (i c) -> p i c", i=NHID),
                      wvz.rearrange("(i p) c -> p i c", p=128))
    colsc_sb = gt_pool.tile([128, 128], F32, tag="colsc", name="colsc")
    nc.sync.dma_start(colsc_sb[:], colsc)
    elup_sb = gt_pool.tile([128, NCOL * 256], BF16, tag="elup", name="elup")
    nc.sync.dma_start(elup_sb[:], elup)
    lamb_sb = gt_pool.tile([128, NCOL * 128], BF16, tag="lamb", name="lamb")
    nc.sync.dma_start(lamb_sb[:], lamb)
    wo_sb = [wpoolP.tile([128, HID], BF16, tag=f"wo{h}", name=f"wo{h}") for h in range(2)]
    for h in range(2):
        nc.sync.dma_start(wo_sb[h][:], wo[128 * h:128 * h + 128, :])
    BETA, BLAM, KTIL, LAMC = 0, 32, 64, 96   # column offsets inside colsc

    # ---------------- Phase A: q/k/v projections (K-contiguous sweeps) ----------------
    with tc.tile_pool(name="pA_ps", bufs=1, space="PSUM") as pA_ps, \
         tc.tile_pool(name="pA_mA", bufs=3, space="PSUM") as pA_mA, \
         tc.tile_pool(name="phaseA_sb", bufs=1) as pA:
        mx = [pA.tile([128, SEQ + 3], BF16, tag=f"mx{g}", name=f"mx{g}") for g in range(4)]
        for g in range(4):
            nc.vector.memset(mx[g][:, 0:3], 0.0)

        pss = [pA_ps.tile([128, 512], F32, tag=f"ps{s}", name=f"ps{s}")
               for s in range(NS4)]

        def sweep(wsl):
            """K-contiguous: for each K-tile i, 4 s-chunk matmuls into 4 fixed
            PSUM banks; stationary loaded once per i."""
            for i in range(NHID):
                w_ap = wsl(i)
                for s in range(NS4):
                    nc.tensor.matmul(pss[s][:], w_ap,
                                     hst_all[:, SEQ * i + 512 * s:SEQ * i + 512 * s + 512],
                                     start=(i == 0), stop=(i == NHID - 1))

        def evac_mx(g):
            for s in range(NS4):
                nc.scalar.copy(mx[g][:, 3 + 512 * s:3 + 512 * s + 512], pss[s][:])

        def conv_macs(g, s4):
            o = 512 * s4
            acc = pA.tile([128, 512], BF16, tag="acc", name="acc", bufs=3)
            nc.vector.tensor_scalar(acc[:], mx[g][:, o:o + 512],
                                    convw_sb[:, 4 * g:4 * g + 1], None, op0=OP.mult)
            for t in range(1, 4):
                nc.vector.scalar_tensor_tensor(acc[:], mx[g][:, o + t:o + t + 512],
                                               convw_sb[:, 4 * g + t:4 * g + t + 1],
                                               acc[:], op0=OP.mult, op1=OP.add)
            return acc

        # PE stream: q, k, v0, v1 sweeps back-to-back; conv/norm elementwise
        # work runs on V/S/G underneath the v sweeps.
        sweep(lambda i: wqk_sb[:, 256 * i:256 * i + 128])
        evac_mx(0)
        co_q, co_k = [], []
        for s4 in range(NS4):
            acc = conv_macs(0, s4)
            co = pA.tile([128, 512], BF16, tag=f"co0_{s4}", name="co", bufs=1)
            nc.scalar.activation(co[:], acc[:], AF.Silu)
            co_q.append(co)
        sweep(lambda i: wqk_sb[:, 256 * i + 128:256 * i + 256])
        evac_mx(1)
        for s4 in range(NS4):
            acc = conv_macs(1, s4)
            co = pA.tile([128, 512], BF16, tag=f"co1_{s4}", name="co", bufs=1)
            nc.scalar.activation(co[:], acc[:], AF.Silu)
            co_k.append(co)
        sweep(lambda i: wvz_sb[:, 512 * i:512 * i + 128])
        evac_mx(2)
        sweep(lambda i: wvz_sb[:, 512 * i + 128:512 * i + 256])
        evac_mx(3)

        # ---- qk l2-norm (ln_exp table set) ----
        for g, cos in ((0, co_q), (1, co_k)):
            ms = pA.tile([2, SEQ], F32, tag="ms", name="ms", bufs=1)
            rstd = pA.tile([2, SEQ], BF16, tag="rstd", name="rstd", bufs=1)
            for s4 in range(NS4):
                sq = pA.tile([128, 512], BF16, tag="sq", name="sq", bufs=2)
                nc.gpsimd.tensor_tensor(sq[:], cos[s4][:], cos[s4][:], op=OP.mult)
                nrm = pA_mA.tile([128, 512], F32, tag="mA", name="mA")
                nc.tensor.matmul(nrm[0:2, :], C["ones_blk"][:], sq[:], start=True, stop=True)
                nc.vector.tensor_scalar(ms[:, 512 * s4:512 * s4 + 512], nrm[0:2, :],
                                        1e-6, None, op0=OP.add)
            nc.scalar.activation(ms[:], ms[:], AF.Ln)
            if g == 0:
                nc.scalar.activation(rstd[:], ms[:], AF.Exp, scale=-0.5, bias=C["qsc"][:])
            else:
                nc.scalar.activation(rstd[:], ms[:], AF.Exp, scale=-0.5)
            # normalize-mult into kqT_all while tiles live (x=1 for q, 0 for k)
            x = 1 - g
            kq4 = kqT_all[:].rearrange("p (n x c) -> p n x c", x=2, c=128)
            for s4 in range(NS4):
                bc = pA_mA.tile([128, 512], F32, tag="mA", name="mA")
                nc.tensor.matmul(bc[:], C["sel2"][:], rstd[:, 512 * s4:512 * s4 + 512],
                                 start=True, stop=True)
                nc.vector.tensor_tensor(
                    kq4[:, 4 * s4:4 * s4 + 4, x, :],
                    bc[:].rearrange("p (t c) -> p t c", c=128),
                    cos[s4][:].rearrange("p (t c) -> p t c", c=128), op=OP.mult)
        for s4 in range(NS4):  # k row layout
            kt = pA_mA.tile([128, 512], BF16, tag="mA", name="mA")
            for j in range(4):
                nn = 4 * s4 + j
                nc.tensor.transpose(kt[:, 128 * j:128 * j + 128],
                                    kqT_all[:, 256 * nn:256 * nn + 128], C["idb"][:])
            nc.scalar.copy(k_rows[:, 512 * s4:512 * s4 + 512], kt[:])

        # ---- v conv (silu) + transpose to row layout ----
        vr = v_rows[:].rearrange("p (t x c) -> p t x c", t=16, x=2)
        for g in (2, 3):
            h = g - 2
            for s4 in range(NS4):
                acc = conv_macs(g, s4)
                co = pA.tile([128, 512], BF16, tag="cov", name="cov", bufs=2)
                nc.scalar.activation(co[:], acc[:], AF.Silu)
                pt = pA_mA.tile([128, 512], BF16, tag="mA", name="mA")
                for j in range(4):
                    nc.tensor.transpose(pt[:, 128 * j:128 * j + 128],
                                        co[:, 128 * j:128 * j + 128], C["idb"][:])
                nc.scalar.copy(vr[:, 4 * s4:4 * s4 + 4, h, :],
                               pt[:].rearrange("p (j c) -> p j c", j=4))

    # ---------------- z sweeps (own PSUM pool, before the chunk pipeline) ----------------
    with tc.tile_pool(name="zp", bufs=1, space="PSUM") as zpool:
        zps = [zpool.tile([128, 512], F32, tag=f"zps{s}", name=f"zps{s}")
               for s in range(2)]
        for (h, sblk) in ((0, 0), (1, 0), (0, 1), (1, 1)):
            for i in range(NHID):
                for s in range(2):
                    blk = 2 * sblk + s
                    nc.tensor.matmul(
                        zps[s][:], wvz_sb[:, 512 * i + 256 + 128 * h:512 * i + 384 + 128 * h],
                        hst_all[:, SEQ * i + 512 * blk:SEQ * i + 512 * blk + 512],
                        start=(i == 0), stop=(i == NHID - 1))
                if i == NHID - 1:
                    for s in range(2):
                        blk = 2 * sblk + s
                        nc.scalar.activation(zT[h][:, 512 * blk:512 * blk + 512],
                                             zps[s][:], AF.Silu)
    ioctx.close()   # free hst/wqk/wvz/convw SBUF for the chunk pipeline

    # ---------------- Phase B: chunks, software-pipelined (v3 structure) ----------------
    sbp = ctx.enter_context(tc.tile_pool(name="chunk_sb", bufs=1))
    stp = ctx.enter_context(tc.tile_pool(name="state", bufs=2))
    gpP = ctx.enter_context(tc.tile_pool(name="gating", bufs=1))
    S_sb = [stp.tile([64, 128], BF16, tag=f"S{h}", name=f"S{h}") for h in range(2)]
    for h in range(2):
        nc.vector.memset(S_sb[h][:], 0.0)

    st = {}  # (n, h) -> dict of tiles

    with tc.tile_pool(name="pB", bufs=1, space="PSUM") as pB:
        bank1 = [pB.tile([128, 512], F32, tag=f"bank1_{h}", name=f"bank1_{h}")
                 for h in range(2)]
        bank2 = [pB.tile([128, 512], F32, tag=f"bank2_{h}", name=f"bank2_{h}")
                 for h in range(2)]
        ser = [pB.tile([128, 512], F32, tag=f"ser{h}", name=f"ser{h}")
               for h in range(2)]
        ptrs = [pB.tile([128, 128], BF16, tag=f"ptr{h}", name=f"ptr{h}")
                for h in range(2)]

        def s1(n, h):
            col = 2 * n + h
            d = st[(n, h)] = {}
            kTs = kqT_all[64 * h:64 * h + 64, 256 * n:256 * n + 128]
            kqs = kqT_all[64 * h:64 * h + 64, 256 * n:256 * n + 256]
            psg = bank1[h][:, 0:256]
            nc.tensor.matmul(psg, kTs, kqs, start=True, stop=True)
            d["psg"] = psg
            krs = k_rows[:, 128 * n + 64 * h:128 * n + 64 * h + 64]
            rhs = sbp.tile([128, 192], BF16, tag=f"rhs{h}", name="rhs", bufs=6)
            nc.scalar.activation(rhs[:, 0:64], krs, AF.Copy,
                                 scale=colsc_sb[:, BLAM + col:BLAM + col + 1])
            nc.scalar.activation(rhs[:, 64:192],
                                 v_rows[:, 256 * n + 128 * h:256 * n + 128 * h + 128],
                                 AF.Copy, scale=colsc_sb[:, BETA + col:BETA + col + 1])
            d["rhs"] = rhs

        def s2(n, h):
            col = 2 * n + h
            d = st[(n, h)]
            amtk = sbp.tile([128, 320], BF16, tag=f"amtk{h}", name="amtk", bufs=8)
            nc.vector.tensor_tensor(amtk[:, 0:256], d["psg"][:],
                                    elup_sb[:, 256 * col:256 * col + 256], op=OP.mult)
            krs = k_rows[:, 128 * n + 64 * h:128 * n + 64 * h + 64]
            nc.scalar.activation(amtk[:, 256:320], krs, AF.Copy,
                                 scale=colsc_sb[:, KTIL + col:KTIL + col + 1])
            d["amtk"] = amtk
            nc.tensor.transpose(ptrs[h][:], amtk[:, 0:128], C["idb"][:])
            d["ptr"] = ptrs[h]

        def s3a(n, h):
            d = st[(n, h)]
            Psb = sbp.tile([128, 128], BF16, tag=f"Psb{h}", name="Psb", bufs=4)
            nc.vector.tensor_tensor(Psb[:], C["idb"][:], d["ptr"][:], op=OP.subtract)
            Bsb = sbp.tile([128, 128], BF16, tag=f"Bsb{h}", name="Bsb", bufs=4)
            nc.gpsimd.tensor_tensor(Bsb[:], C["idb"][:], Psb[:], op=OP.subtract)
            psq = bank1[h][:, 256:384]
            nc.tensor.matmul(psq, d["amtk"][:, 0:128], Bsb[:], start=True, stop=True)
            d["Psb"], d["psq"] = Psb, psq

        def s3b(n, h):
            d = st[(n, h)]
            P1 = sbp.tile([128, 128], BF16, tag=f"P1{h}", name="P1", bufs=4)
            nc.vector.tensor_tensor(P1[:], d["Psb"][:], d["psq"], op=OP.add)
            pwu = bank2[h][:, 0:192]
            nc.tensor.matmul(pwu, P1[:], d["rhs"][:], start=True, stop=True)
            wu = sbp.tile([128, 192], BF16, tag=f"wu{h}", name="wu", bufs=6)
            if h == 0:
                nc.vector.tensor_copy(wu[:], pwu)
            else:
                nc.scalar.copy(wu[:], pwu)
            d["wu"] = wu

        def s4a(n, h):
            col = 2 * n + h
            d = st[(n, h)]
            psm = bank2[h][0:64, 192:384]
            nc.tensor.matmul(psm, d["wu"][:, 0:64], d["amtk"][:, 128:320],
                             start=True, stop=True)
            qlam = sbp.tile([64, 128], BF16, tag=f"qlam{h}", name="qlam", bufs=4)
            nc.gpsimd.tensor_tensor(
                qlam[:], lamb_sb[64 * h:64 * h + 64, 128 * col:128 * col + 128],
                kqT_all[64 * h:64 * h + 64, 256 * n + 128:256 * n + 256], op=OP.mult)
            d["psm"], d["qlam"] = psm, qlam

        def s4b(n, h):
            col = 2 * n + h
            d = st[(n, h)]
            Pt = sbp.tile([64, 128], BF16, tag=f"Pt{h}", name="Pt", bufs=4)
            nc.vector.tensor_tensor(Pt[:], d["qlam"][:], d["psm"][:, 0:128], op=OP.subtract)
            GhT = sbp.tile([64, 64], BF16, tag=f"GhT{h}", name="GhT", bufs=4)
            nc.vector.scalar_tensor_tensor(GhT[:], C["idf"][0:64, 0:64],
                                           colsc_sb[0:64, LAMC + col:LAMC + col + 1],
                                           d["psm"][:, 128:192],
                                           op0=OP.mult, op1=OP.subtract)
            pot = ser[h][:, 0:128]
            nc.tensor.matmul(pot, S_sb[h][:], Pt[:], start=True, stop=False)
            nc.tensor.matmul(pot, d["wu"][:, 64:192], d["amtk"][:, 128:256],
                             start=False, stop=True)
            if h == 0:
                nc.vector.tensor_copy(OT_all[h][:, CH * n:CH * n + CH], pot)
            else:
                nc.scalar.copy(OT_all[h][:, CH * n:CH * n + CH], pot)
            pst = ser[h][0:64, 128:256]
            nc.tensor.matmul(pst, GhT[:], S_sb[h][:], start=True, stop=False)
            nc.tensor.matmul(pst, d["amtk"][:, 256:320], d["wu"][:, 64:192],
                             start=False, stop=True)
            Snew = stp.tile([64, 128], BF16, tag=f"S{h}", name=f"S{h}")
            nc.scalar.copy(Snew[:], pst)
            S_sb[h] = Snew
            del st[(n, h)]

        stages = (s4b, s4a, s3b, s3a, s2, s1)
        for t in range(NCH + len(stages) - 1):
            for k, stage in enumerate(stages):
                n = t - (len(stages) - 1 - k)
                if 0 <= n < NCH:
                    for h in range(2):
                        stage(n, h)

    # ---------------- Phase C: gating + out-proj (batched row DMA) ----------------
    with tc.tile_pool(name="pC_n", bufs=2, space="PSUM") as pC_n, \
         tc.tile_pool(name="pC_o", bufs=4, space="PSUM") as pC_o:
        for s4 in range(NS4):
            sl = slice(512 * s4, 512 * s4 + 512)
            ms4 = gpP.tile([1, 1024], F32, tag="ms4", name="ms4", bufs=2)
            rstd4 = gpP.tile([1, 1024], BF16, tag="rstd4", name="rstd4", bufs=2)
            for h in range(2):
                sq = gpP.tile([128, 512], BF16, tag="sq", name="sq", bufs=2)
                nc.gpsimd.tensor_tensor(sq[:], OT_all[h][:, sl], OT_all[h][:, sl],
                                        op=OP.mult)
                pn = pC_n.tile([128, 512], F32, tag="pn", name="pn")
                nc.tensor.matmul(pn[0:1, :], C["ones_col_h"][:], sq[:],
                                 start=True, stop=True)
                nc.vector.tensor_scalar(ms4[:, 512 * h:512 * h + 512], pn[0:1, :],
                                        1.0 / DV, 1e-6, op0=OP.mult, op1=OP.add)
            nc.scalar.activation(ms4[:], ms4[:], AF.Ln)
            nc.scalar.activation(rstd4[:], ms4[:], AF.Exp, scale=-0.5)
            gated = {}
            for h in range(2):
                pb = pC_n.tile([128, 512], F32, tag="pn", name="pb")
                nc.tensor.matmul(pb[:], C["ones_row"][:], rstd4[:, 512 * h:512 * h + 512],
                                 start=True, stop=True)
                gt = gpP.tile([128, 512], BF16, tag=f"gt{h}", name="gt", bufs=2)
                nc.vector.tensor_tensor(gt[:], OT_all[h][:, sl], pb[:], op=OP.mult)
                nc.gpsimd.tensor_tensor(gt[:], gt[:], zT[h][:, sl], op=OP.mult)
                gated[h] = gt
            for j in range(4):
                s = 4 * s4 + j
                ot = gpP.tile([128, 2048], BF16, tag="ot", name="ot", bufs=3)
                for ho in range(4):
                    po = pC_o.tile([128, 512], F32, tag="po", name="po")
                    for h in range(2):
                        nc.tensor.matmul(po[:], gated[h][:, 128 * j:128 * j + 128],
                                         wo_sb[h][:, 512 * ho:512 * ho + 512],
                                         start=(h == 0), stop=(h == 1))
                    if ho % 2 == 0:
                        nc.vector.tensor_copy(ot[:, 512 * ho:512 * ho + 512], po[:])
                    else:
                        nc.scalar.copy(ot[:, 512 * ho:512 * ho + 512], po[:])
                nc.sync.dma_start(out[128 * s:128 * s + 128, :], ot[:])


def _build_program():
    from contextlib import ExitStack
    nc = bass.Bass("TRN2", target_bir_lowering=False, debug=False)
    hsT = nc.dram_tensor("hsT", [HID, SEQ], BF16, kind="ExternalInput").ap()
    wqk = nc.dram_tensor("wqk", [HID, 256], BF16, kind="ExternalInput").ap()
    wvz = nc.dram_tensor("wvz", [HID, 512], BF16, kind="ExternalInput").ap()
    convw = nc.dram_tensor("convw", [512, 4], F32, kind="ExternalInput").ap()
    wo = nc.dram_tensor("wo", [256, HID], BF16, kind="ExternalInput").ap()
    elup = nc.dram_tensor("elup", [128, NCOL * 256], BF16, kind="ExternalInput").ap()
    lamb = nc.dram_tensor("lamb", [128, NCOL * 128], BF16, kind="ExternalInput").ap()
    colsc = nc.dram_tensor("colsc", [128, 128], F32, kind="ExternalInput").ap()
    out = nc.dram_tensor("out", [SEQ, HID], BF16, kind="ExternalOutput").ap()
    with tile.TileContext(nc) as tc:
        with ExitStack() as ctx:
            _kernel_body(nc, tc, ctx, hsT, wqk, wvz, convw, wo, elup, lamb, colsc, out)
    _split_waits(nc)
    return nc


_PROG = None


def _get_program():
    global _PROG
    if _PROG is None:
        _PROG = _build_program()
    return _PROG


def _shim_ntff_hook():
    """Make bass_utils' `from antenv.axon_hooks import ...` importable."""
    if "antenv.axon_hooks" in sys.modules:
        return
    try:
        import trn_agent_boot.trn_boot as tb
        hook = tb._ntff_profile_via_ctypes("/opt/axon/libaxon_pjrt.so")
    except Exception:
        hook = None
    m = types.ModuleType("antenv.axon_hooks")
    m.get_axon_ntff_profile_hook = lambda: hook
    sys.modules["antenv.axon_hooks"] = m


def _softplus(x):
    return np.logaddexp(0.0, x)


def make_core_inputs(hidden_states, in_proj_qkv, in_proj_a, in_proj_b, in_proj_z,
                     conv_w, A_log, dt_bias, norm_w, out_proj):
    """Host-side sharding: per-core input dicts (core c owns heads 2c, 2c+1).
    Also precomputes, per (chunk, head), the gating/decay tables:
      elup: [A_lower | U_upper] 128x256 blocks (attention-decay matrices)
      lamb: exp(b_j) broadcast rows (128 x 128 per block)
      colsc: per-position column scalars [beta | beta*exp(b) | exp(bC - b) | exp(bC)]
    """
    hs = np.asarray(hidden_states, np.float32)[0]          # (S, HID)
    qkvT = np.ascontiguousarray(np.asarray(in_proj_qkv, np.float32).T)  # (HID, CONV)
    zTw = np.asarray(in_proj_z, np.float32).T              # (HID, VAL)
    cw = np.asarray(conv_w, np.float32)[:, 0, :]           # (CONV, 4)
    A_log = np.asarray(A_log, np.float32)
    dt_bias = np.asarray(dt_bias, np.float32)
    norm_w = np.asarray(norm_w, np.float32)
    op = np.asarray(out_proj, np.float32)                  # (HID, VAL)

    # tiny a/b projections + all decay tables, in float64 on host
    hs64 = hs.astype(np.float64)
    a_full = hs64 @ np.asarray(in_proj_a, np.float64).T    # (S, H)
    b_full = hs64 @ np.asarray(in_proj_b, np.float64).T
    g_full = -np.exp(A_log.astype(np.float64)) * _softplus(a_full + dt_bias)  # (S, H)
    beta_full = 1.0 / (1.0 + np.exp(-b_full))              # (S, H)
    # per-chunk inclusive cumsum of g
    gc = g_full.reshape(NCH, CH, H)
    bcum = np.cumsum(gc, axis=1)                           # (NCH, CH, H)
    betac = beta_full.reshape(NCH, CH, H)

    hsT = np.ascontiguousarray(hs.T).astype(np.float16)    # (HID, S) shared
    pos = np.arange(CH)
    low_mask = pos[:, None] > pos[None, :]                 # j < p strict
    up_mask = pos[:, None] <= pos[None, :]                 # j >= p
    maps = []
    for c in range(8):
        h0, h1 = 2 * c, 2 * c + 1
        qcols = list(range(64 * h0, 64 * h0 + 64)) + list(range(64 * h1, 64 * h1 + 64))
        kcols = [1024 + i for i in qcols]
        vcols0 = list(range(2048 + 128 * h0, 2048 + 128 * h0 + 128))
        vcols1 = list(range(2048 + 128 * h1, 2048 + 128 * h1 + 128))
        wqk = np.ascontiguousarray(qkvT[:, qcols + kcols]).astype(np.float16)
        wvz = np.ascontiguousarray(np.concatenate(
            [qkvT[:, vcols0], qkvT[:, vcols1], zTw[:, 128 * h0:128 * h0 + 128],
             zTw[:, 128 * h1:128 * h1 + 128]], axis=1)).astype(np.float16)
        convw = np.ascontiguousarray(np.concatenate(
            [cw[qcols], cw[kcols], cw[vcols0[0] - 2048 + 2048:vcols0[-1] - 2048 + 2049],
             cw[vcols1[0]:vcols1[-1] + 1]], axis=0))
        wo = np.ascontiguousarray(np.concatenate(
            [op[:, 128 * h0:128 * h0 + 128].T * norm_w[:, None],
             op[:, 128 * h1:128 * h1 + 128].T * norm_w[:, None]],
            axis=0)).astype(np.float16)

        elup = np.zeros((128, NCOL * 256), np.float64)
        lamb = np.zeros((128, NCOL * 128), np.float64)
        colsc = np.zeros((128, 128), np.float64)
        for n in range(NCH):
            for hh, hg in ((0, h0), (1, h1)):
                col = 2 * n + hh
                b = bcum[n, :, hg]                          # (128,)
                beta = betac[n, :, hg]
                # A_lower[p, j] = beta_p * exp(b_p - b_j) for j < p
                # (b decreasing: kept region has b_p - b_j <= 0; clamp the rest)
                A_l = beta[:, None] * np.exp(np.minimum(b[:, None] - b[None, :], 0.0)) * low_mask
                # U_upper[p, j] = exp(b_j - b_p) for j >= p
                U_u = np.exp(np.minimum(b[None, :] - b[:, None], 0.0)) * up_mask
                elup[:, 256 * col:256 * col + 128] = A_l
                elup[:, 256 * col + 128:256 * col + 256] = U_u
                lamb[:, 128 * col:128 * col + 128] = np.exp(b)[None, :]
                colsc[:, col] = beta
                colsc[:, 32 + col] = beta * np.exp(b)
                colsc[:, 64 + col] = np.exp(b[-1] - b)
                colsc[:, 96 + col] = np.exp(b[-1])
        maps.append({"hsT": hsT, "wqk": wqk, "wvz": wvz, "convw": convw, "wo": wo,
                     "elup": elup.astype(np.float16),
                     "lamb": lamb.astype(np.float16),
                     "colsc": colsc.astype(np.float32)})
    return maps


def kernel(hidden_states, in_proj_qkv, in_proj_a, in_proj_b, in_proj_z,
           conv_w, A_log, dt_bias, norm_w, out_proj, is_prefill=1, **_ignored):
    _shim_ntff_hook()
    nc = _get_program()
    maps = make_core_inputs(hidden_states, in_proj_qkv, in_proj_a, in_proj_b,
                            in_proj_z, conv_w, A_log, dt_bias, norm_w, out_proj)
    res = run_bass_kernel_spmd(nc, maps, core_ids=list(range(8)))
    acc = res.results[0]["out"].astype(np.float32)
    for i in range(1, 8):
        acc += res.results[i]["out"].astype(np.float32)
    return acc[None, :, :]
